# revision 11
# baseline (speedup 1.0000x reference)
"""Trainium2 Bass kernel for per-frame complex 5-tap deep-filter FIR.

Problem: spec [8, 3000, 481, 2] f32 complex spectrogram, coef [8, 3000, 96, 10]
per-frame complex FIR coefficients (5 real taps then 5 imag taps) over the
first 96 frequency bins.  out[b,t,f] = sum_k spec[b,t-4+k,f] * coef[b,t,f,k]
(complex, causal zero-padded) for f < 96; bins 96..480 pass through.

Sharding: pure data parallel — batch b -> NeuronCore b (8 batches, 8 cores).

Per-core layout: time tiled as [128 partitions x TS frames/partition] per
tile, NT tiles.  Each partition holds TS frames plus a 4-frame causal halo of
the 96-bin band, contiguously, so every FIR tap is a contiguous free-dim
slice.  Coefficients are loaded in natural layout and read with strided APs.
The 385 pass-through bins never touch SBUF: a DRAM->DRAM DMA copies them.
"""

import numpy as np

B = 8
T = 3000
F = 481
ROW = 2 * F        # 962 floats per frame (interleaved r,i)
NB = 96            # deep-filter band bins
BAND = 2 * NB      # 192 floats per frame of band
NO = 5             # FIR taps
NCOEF = 2 * NO * NB  # 960 floats of coef per frame

TS_LIST = [4, 10, 10]  # frames per partition for each time tile
TP = 128 * sum(TS_LIST)  # padded time (3072)
PAD = 4                # leading zero rows in the DRAM spec (causal halo)

_CACHE = {}


def _build_module():
    import concourse.bass as bass
    import concourse.bacc as bacc
    import concourse.mybir as mybir
    from concourse.tile import TileContext

    f32 = mybir.dt.float32
    mult = mybir.AluOpType.mult
    add = mybir.AluOpType.add
    sub = mybir.AluOpType.subtract
    AP = bass.AP

    nc = bacc.Bacc("TRN2", target_bir_lowering=False, debug=False, num_devices=B)
    # spec carries PAD leading zero rows so the causal halo never underflows:
    # DRAM row r corresponds to frame r - PAD.
    spec_h = nc.dram_tensor("spec", [TP + PAD, ROW], f32, kind="ExternalInput")
    coef_h = nc.dram_tensor("coef", [TP, NCOEF], f32, kind="ExternalInput")
    out_h = nc.dram_tensor("out", [TP, ROW], f32, kind="ExternalOutput")
    spec_ap = spec_h.ap()
    out_ap = out_h.ap()

    with TileContext(nc) as tc:
        with tc.tile_pool(name="pool", bufs=1) as pool:
            base = 0
            for i, TS in enumerate(TS_LIST):
                # Distinct tile names per i -> distinct slots, so all loads
                # can be issued upfront on the sync ring ahead of the
                # pass-through copies.  acc/tmp share slots across tiles
                # (same tag) since DVE work is serial anyway.
                xe = pool.tile([128, (TS + 4) * BAND], f32, name=f"xe{i}")
                cf = pool.tile([128, TS * NCOEF], f32, name=f"cf{i}")
                ob = pool.tile([128, TS * BAND], f32, name=f"ob{i}")
                acc1 = pool.tile([128, max(TS_LIST) * BAND], f32, name="acc1",
                                 tag="acc1")[:, : TS * BAND]
                acc2 = pool.tile([128, max(TS_LIST) * BAND], f32, name="acc2",
                                 tag="acc2")[:, : TS * BAND]
                tmp = pool.tile([128, max(TS_LIST) * BAND], f32, name="tmp",
                                tag="tmp")[:, : TS * BAND]

                # --- load halo-extended band: partition p <- frames
                # [base + p*TS - 4, base + p*TS + TS) x band cols.  The DRAM
                # spec has PAD leading zero rows, so frame t is row t + PAD
                # and the halo never underflows -> one uniform DMA per tile.
                # Band + coef loads ride the sync ring in program order;
                # stores ride the scalar ring so their sem-gates don't
                # FIFO-block later loads or the pass-through.
                nc.sync.dma_start(
                    out=xe[:, :],
                    in_=AP(
                        spec_h,
                        base * ROW,
                        [[TS * ROW, 128], [ROW, TS + 4], [1, BAND]],
                    ),
                )

                # --- load coefficients (contiguous per partition) ---
                nc.sync.dma_start(
                    out=cf[:, :],
                    in_=AP(coef_h, base * NCOEF, [[TS * NCOEF, 128], [1, TS * NCOEF]]),
                )

                # --- complex FIR, pairing (r,i) lanes in each op ---
                cfr = cf.rearrange("p (s f q) -> p s f q", s=TS, f=NB, q=2 * NO)
                part_pair = list(cfr.ap[0])
                sfq = [list(pr) for pr in cfr.ap[1:3]]
                for k in range(NO):
                    u = xe[:, k * BAND : k * BAND + TS * BAND]
                    # c1: (cr_k, ci_k) pairs; c2: (ci_k, cr_k) pairs
                    c1 = cfr[:, :, :, k :: NO]
                    c2 = AP(cf.tensor, cf.offset + NO + k,
                            [part_pair] + sfq + [[-NO, 2]])
                    if k == 0:
                        nc.vector.tensor_tensor(out=acc1[:, :], in0=u, in1=c1, op=mult)
                        nc.vector.tensor_tensor(out=acc2[:, :], in0=u, in1=c2, op=mult)
                    else:
                        nc.vector.tensor_tensor(out=tmp[:, :], in0=u, in1=c1, op=mult)
                        nc.vector.tensor_tensor(
                            out=acc1[:, :], in0=acc1[:, :], in1=tmp[:, :], op=add
                        )
                        nc.vector.tensor_tensor(out=tmp[:, :], in0=u, in1=c2, op=mult)
                        nc.vector.tensor_tensor(
                            out=acc2[:, :], in0=acc2[:, :], in1=tmp[:, :], op=add
                        )

                # fr = even(acc1) - odd(acc1); fi = even(acc2) + odd(acc2)
                a1 = acc1.rearrange("p (s c) -> p s c", c=2)
                a2 = acc2.rearrange("p (s c) -> p s c", c=2)
                obr = ob.rearrange("p (s c) -> p s c", c=2)
                nc.vector.tensor_tensor(
                    out=obr[:, :, 0], in0=a1[:, :, 0], in1=a1[:, :, 1], op=sub
                )
                nc.vector.tensor_tensor(
                    out=obr[:, :, 1], in0=a2[:, :, 0], in1=a2[:, :, 1], op=add
                )

                # --- store band (scalar HWDGE ring, so sem-gated stores
                # don't FIFO-block later loads / pass-through) ---
                nc.scalar.dma_start(
                    out=AP(out_h, base * ROW, [[TS * ROW, 128], [ROW, TS], [1, BAND]]),
                    in_=ob[:, :],
                )
                base += 128 * TS

        # Pass-through bins 96..480: DRAM->DRAM on the sync ring, AFTER all
        # coef loads in program order so it cannot delay them.
        NPT = 8
        for j in range(NPT):
            r0 = j * (TP // NPT)
            r1 = (j + 1) * (TP // NPT)
            nc.sync.dma_start(
                out=out_ap[r0:r1, BAND:ROW],
                in_=spec_ap[PAD + r0 : PAD + r1, BAND:ROW],
            )

    nc.compile()
    return nc


def _get_module():
    if "nc" not in _CACHE:
        _CACHE["nc"] = _build_module()
    return _CACHE["nc"]


def kernel(spec: np.ndarray, coef: np.ndarray) -> np.ndarray:
    from concourse import bass_utils

    assert spec.shape == (B, T, F, 2) and coef.shape == (B, T, NB, 2 * NO)
    spec_p = np.zeros((B, TP + PAD, ROW), np.float32)
    spec_p[:, PAD : PAD + T] = spec.reshape(B, T, ROW)
    coef_p = np.zeros((B, TP, NCOEF), np.float32)
    coef_p[:, :T] = coef.reshape(B, T, NCOEF)

    nc = _get_module()
    in_maps = [{"spec": spec_p[b], "coef": coef_p[b]} for b in range(B)]
    res = bass_utils.run_bass_kernel_spmd(nc, in_maps, core_ids=list(range(B)))
    out = np.empty((B, T, F, 2), np.float32)
    for b in range(B):
        out[b] = res.results[b]["out"][:T].reshape(T, F, 2)
    return out


# revision 14
# speedup vs baseline: 2.1286x; 2.1286x over previous
"""Trainium2 Bass kernel for per-frame complex 5-tap deep-filter FIR.

Problem: spec [8, 3000, 481, 2] f32 complex spectrogram, coef [8, 3000, 96, 10]
per-frame complex FIR coefficients (5 real taps then 5 imag taps) over the
first 96 frequency bins.  out[b,t,f] = sum_k spec[b,t-4+k,f] * coef[b,t,f,k]
(complex, causal zero-padded) for f < 96; bins 96..480 pass through.

Sharding: pure data parallel — batch b -> NeuronCore b (8 batches, 8 cores).

Per-core layout: time tiled as [128 partitions x TS frames/partition] per
tile.  Each partition holds TS frames plus a 4-frame causal halo of the
96-bin band contiguously in the free dim, so every FIR tap is a contiguous
free-dim slice.  Coefficients are loaded in natural layout and read with
strided APs.  The 385 pass-through bins never touch SBUF: a DRAM->DRAM DMA
copies them, queued on the sync HWDGE ring behind the loads; stores ride the
scalar ring so their semaphore gates can't block loads.
"""

import numpy as np

B = 8
T = 3000
F = 481
ROW = 2 * F        # 962 floats per frame (interleaved r,i)
NB = 96            # deep-filter band bins
BAND = 2 * NB      # 192 floats per frame of band
NO = 5             # FIR taps
NCOEF = 2 * NO * NB  # 960 floats of coef per frame

TS_LIST = [4, 10, 10]  # frames per partition for each time tile
TP = 128 * sum(TS_LIST)  # padded time (3072)
PAD = 4                # leading zero rows in the DRAM spec (causal halo)

_CACHE = {}


def _build_module(repeat: int = 1):
    import concourse.bass as bass
    import concourse.bacc as bacc
    import concourse.mybir as mybir
    from concourse.tile import TileContext

    f32 = mybir.dt.float32
    mult = mybir.AluOpType.mult
    add = mybir.AluOpType.add
    sub = mybir.AluOpType.subtract
    AP = bass.AP

    nc = bacc.Bacc("TRN2", target_bir_lowering=False, debug=False, num_devices=B)
    # spec carries PAD leading zero rows so the causal halo never underflows:
    # DRAM row r corresponds to frame r - PAD.
    spec_h = nc.dram_tensor("spec", [TP + PAD, ROW], f32, kind="ExternalInput")
    coef_h = nc.dram_tensor("coef", [TP, NCOEF], f32, kind="ExternalInput")
    out_h = nc.dram_tensor("out", [TP, ROW], f32, kind="ExternalOutput")
    spec_ap = spec_h.ap()
    out_ap = out_h.ap()

    if repeat == 0:
        # I/O-overhead baseline for timing: one trivial DMA, no compute.
        with TileContext(nc) as tc:
            with tc.tile_pool(name="pool", bufs=1) as pool:
                t0 = pool.tile([1, 2], f32)
                nc.sync.dma_start(out=t0[:, :], in_=spec_ap[0:1, 0:2])
                nc.sync.dma_start(out=out_ap[0:1, 0:2], in_=t0[:, :])
        nc.compile()
        return nc

    def emit_body(nc, tc, pool):
        base = 0
        for i, TS in enumerate(TS_LIST):
            # Distinct names per tile i -> distinct slots, so all loads can
            # be issued upfront on the sync ring ahead of the pass-through.
            # acc/tmp share slots across tiles (same tag): DVE is serial.
            xe = pool.tile([128, (TS + 4) * BAND], f32, name=f"xe{i}")
            cf = pool.tile([128, TS * NCOEF], f32, name=f"cf{i}")
            ob = pool.tile([128, TS * BAND], f32, name=f"ob{i}")
            acc1 = pool.tile([128, max(TS_LIST) * BAND], f32, name="acc1",
                             tag="acc1")[:, : TS * BAND]
            acc2 = pool.tile([128, max(TS_LIST) * BAND], f32, name="acc2",
                             tag="acc2")[:, : TS * BAND]
            tmp = pool.tile([128, max(TS_LIST) * BAND], f32, name="tmp",
                            tag="tmp")[:, : TS * BAND]

            # load halo-extended band: partition p <- DRAM rows
            # [base + p*TS, base + p*TS + TS + 4) x band cols (frames
            # shifted by PAD, so this is frames base + p*TS - 4 ...).
            nc.sync.dma_start(
                out=xe[:, :],
                in_=AP(spec_h, base * ROW,
                       [[TS * ROW, 128], [ROW, TS + 4], [1, BAND]]),
            )
            # load coefficients (contiguous per partition)
            nc.sync.dma_start(
                out=cf[:, :],
                in_=AP(coef_h, base * NCOEF,
                       [[TS * NCOEF, 128], [1, TS * NCOEF]]),
            )

            # complex FIR, (r,i) lanes paired in each op
            cfr = cf.rearrange("p (s f q) -> p s f q", s=TS, f=NB, q=2 * NO)
            part_pair = list(cfr.ap[0])
            sfq = [list(pr) for pr in cfr.ap[1:3]]
            for k in range(NO):
                u = xe[:, k * BAND : k * BAND + TS * BAND]
                # c1: (cr_k, ci_k) pairs; c2: (ci_k, cr_k) pairs
                c1 = cfr[:, :, :, k :: NO]
                c2 = AP(cf.tensor, cf.offset + NO + k,
                        [part_pair] + sfq + [[-NO, 2]])
                if k == 0:
                    nc.vector.tensor_tensor(out=acc1[:, :], in0=u, in1=c1, op=mult)
                    nc.vector.tensor_tensor(out=acc2[:, :], in0=u, in1=c2, op=mult)
                else:
                    nc.vector.tensor_tensor(out=tmp[:, :], in0=u, in1=c1, op=mult)
                    nc.vector.tensor_tensor(
                        out=acc1[:, :], in0=acc1[:, :], in1=tmp[:, :], op=add
                    )
                    nc.vector.tensor_tensor(out=tmp[:, :], in0=u, in1=c2, op=mult)
                    nc.vector.tensor_tensor(
                        out=acc2[:, :], in0=acc2[:, :], in1=tmp[:, :], op=add
                    )

            # fr = even(acc1) - odd(acc1); fi = even(acc2) + odd(acc2),
            # written interleaved straight into the store tile
            a1 = acc1.rearrange("p (s c) -> p s c", c=2)
            a2 = acc2.rearrange("p (s c) -> p s c", c=2)
            obr = ob.rearrange("p (s c) -> p s c", c=2)
            nc.vector.tensor_tensor(
                out=obr[:, :, 0], in0=a1[:, :, 0], in1=a1[:, :, 1], op=sub
            )
            nc.vector.tensor_tensor(
                out=obr[:, :, 1], in0=a2[:, :, 0], in1=a2[:, :, 1], op=add
            )

            # store band on the scalar HWDGE ring
            nc.scalar.dma_start(
                out=AP(out_h, base * ROW, [[TS * ROW, 128], [ROW, TS], [1, BAND]]),
                in_=ob[:, :],
            )
            base += 128 * TS

        # Pass-through bins 96..480: DRAM->DRAM on the sync ring AFTER all
        # loads in program order so it cannot delay them.
        NPT = 8
        for j in range(NPT):
            r0 = j * (TP // NPT)
            r1 = (j + 1) * (TP // NPT)
            nc.sync.dma_start(
                out=out_ap[r0:r1, BAND:ROW],
                in_=spec_ap[PAD + r0 : PAD + r1, BAND:ROW],
            )

    with TileContext(nc) as tc:
        with tc.tile_pool(name="pool", bufs=1) as pool:
            for _ in range(repeat):
                emit_body(nc, tc, pool)

    nc.compile()
    return nc


def _get_module(repeat: int = 1):
    if repeat not in _CACHE:
        _CACHE[repeat] = _build_module(repeat)
    return _CACHE[repeat]


def kernel(spec: np.ndarray, coef: np.ndarray) -> np.ndarray:
    from concourse import bass_utils

    assert spec.shape == (B, T, F, 2) and coef.shape == (B, T, NB, 2 * NO)
    spec_p = np.zeros((B, TP + PAD, ROW), np.float32)
    spec_p[:, PAD : PAD + T] = spec.reshape(B, T, ROW)
    coef_p = np.zeros((B, TP, NCOEF), np.float32)
    coef_p[:, :T] = coef.reshape(B, T, NCOEF)

    nc = _get_module()
    in_maps = [{"spec": spec_p[b], "coef": coef_p[b]} for b in range(B)]
    res = bass_utils.run_bass_kernel_spmd(nc, in_maps, core_ids=list(range(B)))
    out = np.empty((B, T, F, 2), np.float32)
    for b in range(B):
        out[b] = res.results[b]["out"][:T].reshape(T, F, 2)
    return out
